# revision 1
# baseline (speedup 1.0000x reference)
"""BinCalibrationContributionLoss kernel for 8 Trainium2 NeuronCores.

Math: the reference loss
    loss = mean_i [ -(1 + g*(orig_b(i) - updated_i)) * picked_i ]
collapses exactly onto 15-bin segment sums.  With
    conf_i = exp(m_i - log s_i),  s_i = sum_j exp(x_ij),  m_i = max_j x_ij,
    t_i = x[i, y_i],  acc_i = (t_i == m_i),  picked_i = t_i - log s_i,
    d_i = conf_i - acc_i
and per-bin sums over samples  CNT, SC (conf), SA (acc), SP (picked),
SPD (picked*d):
    A_b    = SC_b - SA_b
    orig_b = |A_b| / max(CNT_b, 1)
    w_b    = [CNT_b > 1] / max(CNT_b - 1, 1)
    sum_i updated_i*picked_i = sum_b w_b * sign(A_b) * (A_b*SP_b - SPD_b)
      (exact whenever |A_b| > 1 -- always in practice; validated vs the
       reference at ~1e-7 rel on the full problem)
    loss = -[ sum_b SP_b + g*( sum_b orig_b*SP_b
              - sum_b w_b*sign(A_b)*(A_b*SP_b - SPD_b) ) ] / N

Device work per core (125000 rows, data-parallel over 8 cores): stream x
in 62 tiles of [128 partitions x 16 rows x 100 classes] (819 KB DMAs);
exp on ScalarE, a 2-level pairwise sum tree on GpSimd, segmented
sum/max reduces on VectorE, small per-sample ops chunked, and a per-tile
PE matmul (bf16) accumulating the [15,5] bin table into PSUM.
t = x[i, y_i] is host input prep (TRN2 has no per-partition gather op).
"""

import numpy as np

import concourse.bass as bass
import concourse.tile as tile
from concourse import bacc, mybir
from concourse.bass_utils import run_bass_kernel_spmd

# ---- problem constants ----
N_TOTAL = 1_000_000
C = 100
N_CORES = 8
R = N_TOTAL // N_CORES          # 125000 rows per core
G = 16                          # rows per partition per tile
TILE_ROWS = 128 * G             # 2048
T_MAIN = R // TILE_ROWS         # 61 full tiles -> 124928 rows
MAIN_ROWS = T_MAIN * TILE_ROWS
TAIL_ROWS = R - MAIN_ROWS       # 72
T_ALL = T_MAIN + 1              # 62 tiles (last is host-padded tail)
COLS = T_ALL * G                # 992 sample-columns per partition
NUM_BINS = 15
GAMMA = 0.047
CHUNK_TILES = 16                # small-op batching: 16 tiles = 256 columns
F32 = mybir.dt.float32
BF16 = mybir.dt.bfloat16
I32 = mybir.dt.int32

_CACHED_NC = None


def _patch_act_tables():
    """Force Exp and Ln to resolve to the combined table set so the
    ScalarE never swaps tables mid-kernel (~1.3us per swap otherwise).
    Set membership is edited in place; set order (and hence ids) is kept."""
    from concourse import bacc as _bacc_mod
    if getattr(_bacc_mod, "_ant_act_tables_patched", False):
        return
    from concourse.hw_specs import get_activation_tables as _orig

    def _patched(arch):
        t = _orig(arch)
        combined = "natural_log_exp_and_others"
        if combined in t:
            both = {mybir.ActivationFunctionType.Exp,
                    mybir.ActivationFunctionType.Ln}
            for name, fns in t.items():
                if name != combined:
                    fns -= both
        return t

    _bacc_mod.get_activation_tables = _patched
    _bacc_mod._ant_act_tables_patched = True


def build_nc(t_main=T_MAIN):
    """Build the single-core Bass program (SPMD across 8 cores)."""
    _patch_act_tables()
    t_all = t_main + 1
    cols = t_all * G
    nc = bacc.Bacc("TRN2", target_bir_lowering=False, debug=False)
    x_in = nc.dram_tensor("x", [t_main, 128, G * C], F32, kind="ExternalInput")
    xt_in = nc.dram_tensor("xt", [1, 128, G * C], F32, kind="ExternalInput")
    tg_in = nc.dram_tensor("tg", [128, cols], F32, kind="ExternalInput")
    iot_in = nc.dram_tensor("iot", [128, NUM_BINS], I32, kind="ExternalInput")
    mk_in = nc.dram_tensor("mk", [128, G], BF16, kind="ExternalInput")
    out_d = nc.dram_tensor("out", [8 * NUM_BINS, 5, 8], F32,
                           kind="ExternalOutput")

    MG = 8  # groups per matmul (lhsT M = MG*15 = 120 <= 128)

    with tile.TileContext(nc) as tc:
        with (
            tc.tile_pool(name="xp", bufs=3) as xp,
            tc.tile_pool(name="ep", bufs=3) as ep,
            tc.tile_pool(name="shp", bufs=3) as shp,
            tc.tile_pool(name="ohp", bufs=2) as ohp,
            tc.tile_pool(name="arr", bufs=1) as arr,
            tc.tile_pool(name="psum", bufs=1, space="PSUM") as psp,
        ):
            t_arr = arr.tile([128, cols], F32, tag="t_arr")
            m_arr = arr.tile([128, cols], F32, tag="m_arr")
            s_arr = arr.tile([128, cols], F32, tag="s_arr")
            logs_a = arr.tile([128, cols], F32, tag="logs")
            u_arr = arr.tile([128, cols], F32, tag="u")
            d_arr = arr.tile([128, cols], BF16, tag="d")
            bini = arr.tile([128, cols], I32, tag="bini")
            vals = arr.tile([128, 5, cols], BF16, tag="vals")
            iot = arr.tile([128, NUM_BINS], I32, tag="iot")
            mk = arr.tile([128, G], BF16, tag="mk")
            acc_ps = psp.tile([8 * NUM_BINS, 5, 8], F32, tag="acc")
            outs = arr.tile([8 * NUM_BINS, 5, 8], F32, tag="outs")

            nc.sync.dma_start(t_arr[:], tg_in[:])
            nc.sync.dma_start(iot[:], iot_in[:])
            nc.sync.dma_start(mk[:], mk_in[:])

            # count plane = 1 for valid samples (pad zeroed via mask)
            nc.gpsimd.memset(vals[:, 0, :], 1.0)

            op = mybir.AluOpType
            afn = mybir.ActivationFunctionType
            ax = mybir.AxisListType

            def load_compute_tile(t):
                src = x_in[t] if t < t_main else xt_in[0]
                xt_t = xp.tile([128, G, C], F32, tag="x")
                nc.sync.dma_start(xt_t[:], src)
                e_t = ep.tile([128, G, C], F32, tag="e")
                nc.scalar.activation(e_t[:], xt_t[:], afn.Exp)
                sl = slice(G * t, G * (t + 1))
                # 2-level pairwise sum tree on GpSimd, final reduce on DVE
                eh1 = shp.tile([128, G, 50], F32, tag="eh1")
                nc.gpsimd.tensor_tensor(
                    eh1[:], e_t[:, :, 0:50], e_t[:, :, 50:100], op.add)
                eh2 = shp.tile([128, G, 25], F32, tag="eh2")
                nc.gpsimd.tensor_tensor(
                    eh2[:], eh1[:, :, 0:25], eh1[:, :, 25:50], op.add)
                nc.vector.reduce_sum(s_arr[:, sl], eh2[:], axis=ax.X)
                nc.vector.reduce_max(m_arr[:, sl], xt_t[:], axis=ax.X)

            def chunk_smalls(c0, c1):
                cs = slice(c0, c1)
                nc.scalar.activation(logs_a[:, cs], s_arr[:, cs], afn.Ln)
                # picked = t - log s  (bf16 plane)
                nc.vector.tensor_tensor(
                    vals[:, 3, cs], t_arr[:, cs], logs_a[:, cs], op.subtract)
                # u = m - log s ; conf = exp(u)
                nc.vector.tensor_tensor(
                    u_arr[:, cs], m_arr[:, cs], logs_a[:, cs], op.subtract)
                nc.scalar.activation(vals[:, 1, cs], u_arr[:, cs], afn.Exp)
                # acc = (t == m)
                nc.vector.tensor_tensor(
                    vals[:, 2, cs], t_arr[:, cs], m_arr[:, cs], op.is_equal)
                # d = conf - acc ; pd = picked * d
                nc.vector.tensor_tensor(
                    d_arr[:, cs], vals[:, 1, cs], vals[:, 2, cs], op.subtract)
                nc.vector.tensor_tensor(
                    vals[:, 4, cs], vals[:, 3, cs], d_arr[:, cs], op.mult)
                # bin index: trunc(min(conf*15, 14.49)) -> int32
                nc.vector.tensor_scalar(
                    bini[:, cs], vals[:, 1, cs], 15.0, 14.49, op.mult, op.min)

            def bin_matmuls(tiles, oh_chunk, c0):
                for t in tiles:
                    for h in range(G // MG):
                        lo = G * t - c0 + MG * h
                        nc.tensor.matmul(
                            acc_ps[:],
                            oh_chunk[:, lo:lo + MG, :],
                            vals[:, :, G * t + MG * h:G * t + MG * (h + 1)],
                            start=(t == 0 and h == 0),
                            stop=(t == t_all - 1 and h == G // MG - 1),
                        )

            n_chunks = (t_all + CHUNK_TILES - 1) // CHUNK_TILES
            for ch in range(n_chunks):
                tiles = range(ch * CHUNK_TILES, min((ch + 1) * CHUNK_TILES,
                                                    t_all))
                for t in tiles:
                    load_compute_tile(t)
                c0, c1 = G * tiles[0], G * (tiles[-1] + 1)
                w = c1 - c0
                chunk_smalls(c0, c1)
                if ch == n_chunks - 1:
                    # zero pad samples (rows >= TAIL_ROWS of the tail tile)
                    tl = slice(G * t_main, cols)
                    mkb = mk[:, None, :].broadcast_to([128, 5, G])
                    nc.vector.tensor_tensor(
                        vals[:, :, tl], vals[:, :, tl], mkb, op.mult)
                # bin one-hot: (bini == b), [128, w, 15] bf16
                ohj = ohp.tile([128, CHUNK_TILES * G, NUM_BINS], BF16,
                               tag="ohj")
                binb = bini[:, c0:c1][:, :, None].broadcast_to(
                    [128, w, NUM_BINS])
                iotb = iot[:, None, :].broadcast_to([128, w, NUM_BINS])
                nc.vector.tensor_tensor(ohj[:, :w, :], binb, iotb, op.is_equal)
                bin_matmuls(tiles, ohj, c0)

            nc.vector.tensor_copy(outs[:], acc_ps[:])
            nc.sync.dma_start(out_d[:], outs[:])

    nc.finalize()
    return nc


def _iota_tile():
    row = np.arange(NUM_BINS, dtype=np.int32)
    return np.broadcast_to(row, (128, NUM_BINS)).copy()


def _tail_mask():
    rows = np.arange(TILE_ROWS) < TAIL_ROWS
    import ml_dtypes
    return rows.reshape(128, G).astype(ml_dtypes.bfloat16)


def _layout_cols(vec, cols=COLS, t_main=T_MAIN):
    """Map a per-core [R] vector to the on-chip [128, cols] layout.

    Sample at (tile T, partition p, group g) is row T*2048 + 16*p + g and
    lives at column 16*T + g."""
    main_rows = t_main * TILE_ROWS
    out = np.zeros((128, cols), dtype=vec.dtype)
    main = vec[:main_rows].reshape(t_main, 128, G)
    out[:, :t_main * G] = np.transpose(main, (1, 0, 2)).reshape(128, t_main * G)
    tail = np.zeros(TILE_ROWS, dtype=vec.dtype)
    tail[:vec.shape[0] - main_rows] = vec[main_rows:]
    out[:, t_main * G:] = tail.reshape(128, G)
    return out


def _host_finish(tables):
    """tables: [cores, 120, 5, 8] -> scalar loss (f64 internally)."""
    t = np.asarray(tables, dtype=np.float64)
    tab = np.zeros((NUM_BINS, 5))
    for g in range(8):
        tab += t[:, g * NUM_BINS:(g + 1) * NUM_BINS, :, g].sum(axis=0)
    cnt, sc, sa, sp, spd = tab[:, 0], tab[:, 1], tab[:, 2], tab[:, 3], tab[:, 4]
    a = sc - sa
    orig = np.abs(a) / np.maximum(cnt, 1.0)
    w = (cnt > 1.0) / np.maximum(cnt - 1.0, 1.0)
    upd = (w * np.sign(a) * (a * sp - spd)).sum()
    loss = -(sp.sum() + GAMMA * ((orig * sp).sum() - upd)) / N_TOTAL
    return np.float32(loss)


def make_in_maps(x, y):
    x = np.ascontiguousarray(np.asarray(x, dtype=np.float32))
    tvec = x[np.arange(x.shape[0]), np.asarray(y).astype(np.int64)]
    tvec = tvec.astype(np.float32)
    iot = _iota_tile()
    mkt = _tail_mask()
    in_maps = []
    for c in range(N_CORES):
        r0 = c * R
        xm = x[r0:r0 + MAIN_ROWS].reshape(T_MAIN, 128, G * C)
        xt = np.zeros((TILE_ROWS, C), dtype=np.float32)
        xt[:TAIL_ROWS] = x[r0 + MAIN_ROWS:r0 + R]
        xt = xt.reshape(1, 128, G * C)
        tg = _layout_cols(tvec[r0:r0 + R])
        in_maps.append({"x": xm, "xt": xt, "tg": tg, "iot": iot, "mk": mkt})
    return in_maps


def kernel(x, y):
    global _CACHED_NC
    x = np.asarray(x)
    assert x.shape == (N_TOTAL, C)
    in_maps = make_in_maps(x, y)
    if _CACHED_NC is None:
        _CACHED_NC = build_nc()
    res = run_bass_kernel_spmd(_CACHED_NC, in_maps,
                               core_ids=list(range(N_CORES)))
    tables = [res.results[c]["out"] for c in range(N_CORES)]
    return _host_finish(tables)


if __name__ == "__main__":
    rng = np.random.default_rng(0)
    x = rng.standard_normal((N_TOTAL, C), dtype=np.float32)
    y = rng.integers(0, C, N_TOTAL).astype(np.int64)
    print("loss:", kernel(x, y))



# revision 9
# speedup vs baseline: 1.2900x; 1.2900x over previous
"""BinCalibrationContributionLoss kernel for 8 Trainium2 NeuronCores.

Math: the reference loss
    loss = mean_i [ -(1 + g*(orig_b(i) - updated_i)) * picked_i ]
collapses exactly onto 15-bin segment sums.  With
    conf_i = max_j p_ij,  s_i = sum_j exp(x_ij),  m_i = max_j x_ij,
    t_i = x[i, y_i],  acc_i = (t_i == m_i),  picked_i = t_i - log s_i,
    d_i = conf_i - acc_i
and per-bin sums over samples  CNT, SC (conf), SA (acc), SP (picked),
SPD (picked*d):
    A_b    = SC_b - SA_b
    orig_b = |A_b| / max(CNT_b, 1)
    w_b    = [CNT_b > 1] / max(CNT_b - 1, 1)
    sum_i updated_i*picked_i = sum_b w_b * sign(A_b) * (A_b*SP_b - SPD_b)
      (exact whenever |A_b| > 1 -- always in practice; validated vs the
       reference at ~1e-7 rel on the full problem)
    loss = -[ sum_b SP_b + g*( sum_b orig_b*SP_b
              - sum_b w_b*sign(A_b)*(A_b*SP_b - SPD_b) ) ] / N

Device mapping choices (engine-balance driven; DVE is the scarce engine):
 * max is computed in EXP space: m_e = max_j exp(x_ij) = exp(m_i), so the
   max tree runs on the bf16 exp tile in the DVE 2x packed mode.
   conf = m_e * (1/s) via reciprocal_approx_fast; acc = (t_e >= m_e) with
   t_e = bf16(exp(t)) prepared on host (np.exp vs ScalarE spline differ by
   ~2ULP; collision probability ~3e-5/sample, loss impact ~1e-5 rel).
 * Bin membership is accumulated CUMULATIVELY: cum_b = [conf > edge_b]
   (edge_b = b/15) in a [15, cols] layout against a pre-materialized
   [15, w] edge tile so the compare runs in DVE 2x mode.  The host
   recovers per-bin tables as tab_b = cum_b - cum_{b+1}.  No clipping is
   needed: conf rounding to exactly 1.0 still lands in bin 14.
 * sum tree level 1 (adds) is split GpSimd/DVE; GpSimd only implements
   add/sub/mult tensor ops.
 * final 25->1 reduces stay on DVE (tensor_reduce has no 2x mode).

Device work per core (125000 rows, data-parallel over 8 cores): stream x
in 31 tiles of [128 partitions x 32 rows x 100 classes].
t = x[i, y_i] is host input prep (TRN2 has no per-partition gather op).

Two input modes: xdt="f32" (DMA-bound, exact) or "bf16" (host casts x to
bf16, halving HBM traffic; t gathered from the bf16 copy; expected extra
loss error ~1e-4 rel, tolerance is 2e-2).
"""

import numpy as np

import concourse.bass as bass
import concourse.tile as tile
from concourse import bacc, mybir
from concourse.bass_utils import run_bass_kernel_spmd

# ---- problem constants ----
N_TOTAL = 1_000_000
C = 100
N_CORES = 8
R = N_TOTAL // N_CORES          # 125000 rows per core
G = 32                          # rows per partition per tile
TILE_ROWS = 128 * G             # 4096
T_MAIN = R // TILE_ROWS         # 30 full tiles -> 122880 rows
MAIN_ROWS = T_MAIN * TILE_ROWS
TAIL_ROWS = R - MAIN_ROWS       # 2120
T_ALL = T_MAIN + 1              # 31 tiles (last is host-padded tail)
COLS = T_ALL * G                # 992 sample-columns per partition
NUM_BINS = 15
GAMMA = 0.047
CHUNK_TILES = 8                 # small-op batching: 8 tiles = 256 columns
CHUNK_W = CHUNK_TILES * G       # 256
MG = 8                          # column-groups per matmul (M = 15*8 = 120)
EH1_GS = 30                     # sum-tree L1 columns on GpSimd (of 50)
F32 = mybir.dt.float32
BF16 = mybir.dt.bfloat16

XDT_DEFAULT = "bf16"
_CACHED = {}


def _patch_act_tables():
    """Force Exp and Ln to resolve to the combined table set so the
    ScalarE never swaps tables mid-kernel (~1.3us per swap otherwise)."""
    from concourse import bacc as _bacc_mod
    if getattr(_bacc_mod, "_ant_act_tables_patched", False):
        return
    from concourse.hw_specs import get_activation_tables as _orig

    def _patched(arch):
        t = _orig(arch)
        combined = "natural_log_exp_and_others"
        if combined in t:
            both = {mybir.ActivationFunctionType.Exp,
                    mybir.ActivationFunctionType.Ln}
            for name, fns in t.items():
                if name != combined:
                    fns -= both
        return t

    _bacc_mod.get_activation_tables = _patched
    _bacc_mod._ant_act_tables_patched = True


def build_nc(xdt=XDT_DEFAULT, t_main=T_MAIN):
    """Build the single-core Bass program (SPMD across 8 cores)."""
    _patch_act_tables()
    XDT = F32 if xdt == "f32" else BF16
    t_all = t_main + 1
    cols = t_all * G
    nc = bacc.Bacc("TRN2", target_bir_lowering=False, debug=False)
    x_in = nc.dram_tensor("x", [t_main, 128, G * C], XDT, kind="ExternalInput")
    xt_in = nc.dram_tensor("xt", [1, 128, G * C], XDT, kind="ExternalInput")
    tg_in = nc.dram_tensor("tg", [128, cols], F32, kind="ExternalInput")
    te_in = nc.dram_tensor("te", [128, cols], BF16, kind="ExternalInput")
    edg_in = nc.dram_tensor("edg", [128, NUM_BINS], F32, kind="ExternalInput")
    mk_in = nc.dram_tensor("mk", [128, G], BF16, kind="ExternalInput")
    out_d = nc.dram_tensor("out", [NUM_BINS * MG, 5, MG], F32,
                           kind="ExternalOutput")

    with tile.TileContext(nc) as tc:
        with (
            tc.tile_pool(name="xp", bufs=3) as xp,
            tc.tile_pool(name="ep", bufs=3) as ep,
            tc.tile_pool(name="shp", bufs=3) as shp,
            tc.tile_pool(name="ohp", bufs=2) as ohp,
            tc.tile_pool(name="arr", bufs=1) as arr,
            tc.tile_pool(name="psum", bufs=1, space="PSUM") as psp,
        ):
            t_arr = arr.tile([128, cols], F32, tag="t_arr")
            te_arr = arr.tile([128, cols], BF16, tag="te_arr")
            m_arr = arr.tile([128, cols], BF16, tag="m_arr")   # m_e = exp(m)
            s_arr = arr.tile([128, cols], F32, tag="s_arr")
            rs_arr = arr.tile([128, cols], F32, tag="rs_arr")  # 1/s
            logs_a = arr.tile([128, cols], F32, tag="logs")
            d_arr = arr.tile([128, cols], BF16, tag="d")
            vals = arr.tile([128, 5, cols], BF16, tag="vals")
            edg = arr.tile([128, NUM_BINS], F32, tag="edg")
            mk = arr.tile([128, G], BF16, tag="mk")
            acc_ps = psp.tile([NUM_BINS * MG, 5, MG], F32, tag="acc")
            outs = arr.tile([NUM_BINS * MG, 5, MG], F32, tag="outs")

            nc.sync.dma_start(t_arr[:], tg_in[:])
            nc.sync.dma_start(te_arr[:], te_in[:])
            nc.sync.dma_start(edg[:], edg_in[:])
            nc.sync.dma_start(mk[:], mk_in[:])

            # count plane = 1 for valid samples (pad zeroed via mask)
            nc.gpsimd.memset(vals[:, 0, :], 1.0)

            op = mybir.AluOpType
            afn = mybir.ActivationFunctionType
            ax = mybir.AxisListType

            def load_compute_tile(t):
                src = x_in[t] if t < t_main else xt_in[0]
                xt_t = xp.tile([128, G, C], XDT, tag="x")
                nc.sync.dma_start(xt_t[:], src)
                e_t = ep.tile([128, G, C], BF16, tag="e")
                nc.scalar.activation(e_t[:], xt_t[:], afn.Exp)
                sl = slice(G * t, G * (t + 1))
                # sum tree: L1 split GpSimd/DVE (adds), L2 + reduce on DVE
                eh1 = shp.tile([128, G, 50], BF16, tag="eh1")
                nc.gpsimd.tensor_tensor(
                    eh1[:, :, 0:EH1_GS], e_t[:, :, 0:EH1_GS],
                    e_t[:, :, 50:50 + EH1_GS], op.add)
                nc.vector.tensor_tensor(
                    eh1[:, :, EH1_GS:50], e_t[:, :, EH1_GS:50],
                    e_t[:, :, 50 + EH1_GS:100], op.add)
                eh2 = shp.tile([128, G, 25], BF16, tag="eh2")
                nc.vector.tensor_tensor(
                    eh2[:], eh1[:, :, 0:25], eh1[:, :, 25:50], op.add)
                nc.vector.reduce_sum(s_arr[:, sl], eh2[:], axis=ax.X)
                # max tree in exp space (bf16 2x on DVE)
                mh1 = shp.tile([128, G, 50], BF16, tag="mh1")
                nc.vector.tensor_tensor(
                    mh1[:], e_t[:, :, 0:50], e_t[:, :, 50:100], op.max)
                mh2 = shp.tile([128, G, 25], BF16, tag="mh2")
                nc.vector.tensor_tensor(
                    mh2[:], mh1[:, :, 0:25], mh1[:, :, 25:50], op.max)
                nc.vector.reduce_max(m_arr[:, sl], mh2[:], axis=ax.X)

            def chunk_smalls(c0, c1):
                cs = slice(c0, c1)
                nc.scalar.activation(logs_a[:, cs], s_arr[:, cs], afn.Ln)
                nc.vector.reciprocal_approx_fast(rs_arr[:, cs], s_arr[:, cs])
                # picked = t - log s  (bf16 plane)
                nc.vector.tensor_tensor(
                    vals[:, 3, cs], t_arr[:, cs], logs_a[:, cs], op.subtract)
                # conf = m_e / s
                nc.vector.tensor_tensor(
                    vals[:, 1, cs], m_arr[:, cs], rs_arr[:, cs], op.mult)
                # acc = (t_e >= m_e)
                nc.vector.tensor_tensor(
                    vals[:, 2, cs], te_arr[:, cs], m_arr[:, cs], op.is_ge)
                # d = conf - acc ; pd = picked * d
                nc.vector.tensor_tensor(
                    d_arr[:, cs], vals[:, 1, cs], vals[:, 2, cs], op.subtract)
                nc.vector.tensor_tensor(
                    vals[:, 4, cs], vals[:, 3, cs], d_arr[:, cs], op.mult)

            def bin_matmuls(tiles, cum, c0, t_last):
                for t in tiles:
                    for h in range(G // MG):
                        lo = G * t - c0 + MG * h
                        nc.tensor.matmul(
                            acc_ps[:],
                            cum[:, lo:lo + MG, :],
                            vals[:, :, G * t + MG * h:G * t + MG * (h + 1)],
                            start=(t == 0 and h == 0),
                            stop=(t == t_last and h == G // MG - 1),
                        )

            n_chunks = (t_all + CHUNK_TILES - 1) // CHUNK_TILES
            for ch in range(n_chunks):
                tiles = range(ch * CHUNK_TILES, min((ch + 1) * CHUNK_TILES,
                                                    t_all))
                for t in tiles:
                    load_compute_tile(t)
                c0, c1 = G * tiles[0], G * (tiles[-1] + 1)
                w = c1 - c0
                chunk_smalls(c0, c1)
                if ch == n_chunks - 1:
                    # zero pad samples (rows >= TAIL_ROWS of the tail tile)
                    tl = slice(G * t_main, cols)
                    mkb = mk[:, None, :].broadcast_to([128, 5, G])
                    nc.vector.tensor_tensor(
                        vals[:, :, tl], vals[:, :, tl], mkb, op.mult)
                # cumulative bin flags: cum[col, b] = (conf > edge_b)
                cum = ohp.tile([128, CHUNK_W, NUM_BINS], BF16, tag="cum")
                confb = vals[:, 1, c0:c1][:, :, None].broadcast_to(
                    [128, w, NUM_BINS])
                edgb = edg[:, None, :].broadcast_to([128, w, NUM_BINS])
                nc.vector.tensor_tensor(cum[:, :w, :], confb, edgb, op.is_gt)
                bin_matmuls(tiles, cum, c0, t_all - 1)

            nc.vector.tensor_copy(outs[:], acc_ps[:])
            nc.sync.dma_start(out_d[:], outs[:])

    nc.finalize()
    return nc


def _edge_tile():
    """[128, 15] f32: lower bin edges, linspace(0,1,16)[b]."""
    row = np.linspace(0.0, 1.0, NUM_BINS + 1)[:NUM_BINS].astype(np.float32)
    return np.broadcast_to(row, (128, NUM_BINS)).copy()


def _tail_mask():
    rows = np.arange(TILE_ROWS) < TAIL_ROWS
    import ml_dtypes
    return rows.reshape(128, G).astype(ml_dtypes.bfloat16)


def _layout_cols(vec, cols=COLS, t_main=T_MAIN):
    """Map a per-core [R] vector to the on-chip [128, cols] layout.

    Sample at (tile T, partition p, group g) is row T*4096 + 32*p + g and
    lives at column 32*T + g."""
    main_rows = t_main * TILE_ROWS
    out = np.zeros((128, cols), dtype=vec.dtype)
    main = vec[:main_rows].reshape(t_main, 128, G)
    out[:, :t_main * G] = np.transpose(main, (1, 0, 2)).reshape(128, t_main * G)
    tail = np.zeros(TILE_ROWS, dtype=vec.dtype)
    tail[:vec.shape[0] - main_rows] = vec[main_rows:]
    out[:, t_main * G:] = tail.reshape(128, G)
    return out


def _host_finish(tables):
    """tables: [cores, 120, 5, 8] cumulative-bin sums -> scalar loss.

    Matmul M index is group-major: row = g*15 + b, diagonal col g."""
    t = np.asarray(tables, dtype=np.float64)
    cum = np.zeros((NUM_BINS, 5))
    for g in range(MG):
        cum += t[:, g * NUM_BINS:(g + 1) * NUM_BINS, :, g].sum(axis=0)
    tab = cum.copy()
    tab[:-1] -= cum[1:]
    cnt, sc, sa, sp, spd = tab[:, 0], tab[:, 1], tab[:, 2], tab[:, 3], tab[:, 4]
    a = sc - sa
    orig = np.abs(a) / np.maximum(cnt, 1.0)
    w = (cnt > 1.0) / np.maximum(cnt - 1.0, 1.0)
    upd = (w * np.sign(a) * (a * sp - spd)).sum()
    loss = -(sp.sum() + GAMMA * ((orig * sp).sum() - upd)) / N_TOTAL
    return np.float32(loss)


def make_in_maps(x, y, xdt=XDT_DEFAULT):
    import ml_dtypes
    x = np.ascontiguousarray(np.asarray(x, dtype=np.float32))
    if xdt == "bf16":
        xs = x.astype(ml_dtypes.bfloat16)
    else:
        xs = x
    tvec = xs[np.arange(x.shape[0]), np.asarray(y).astype(np.int64)]
    tvec = tvec.astype(np.float32)
    te = np.exp(tvec).astype(ml_dtypes.bfloat16)
    edg = _edge_tile()
    mkt = _tail_mask()
    in_maps = []
    for c in range(N_CORES):
        r0 = c * R
        xm = xs[r0:r0 + MAIN_ROWS].reshape(T_MAIN, 128, G * C)
        xt = np.zeros((TILE_ROWS, C), dtype=xs.dtype)
        xt[:TAIL_ROWS] = xs[r0 + MAIN_ROWS:r0 + R]
        xt = xt.reshape(1, 128, G * C)
        tg = _layout_cols(tvec[r0:r0 + R])
        teg = _layout_cols(te[r0:r0 + R])
        in_maps.append({"x": xm, "xt": xt, "tg": tg, "te": teg,
                       "edg": edg, "mk": mkt})
    return in_maps


def kernel(x, y):
    x = np.asarray(x)
    assert x.shape == (N_TOTAL, C)
    xdt = XDT_DEFAULT
    in_maps = make_in_maps(x, y, xdt)
    if xdt not in _CACHED:
        _CACHED[xdt] = build_nc(xdt)
    res = run_bass_kernel_spmd(_CACHED[xdt], in_maps,
                               core_ids=list(range(N_CORES)))
    tables = [res.results[c]["out"] for c in range(N_CORES)]
    return _host_finish(tables)


if __name__ == "__main__":
    rng = np.random.default_rng(0)
    x = rng.standard_normal((N_TOTAL, C), dtype=np.float32)
    y = rng.integers(0, C, N_TOTAL).astype(np.int64)
    print("loss:", kernel(x, y))
